# revision 34
# baseline (speedup 1.0000x reference)
"""ClusterKLLoss Trainium2 kernel (8 NeuronCores, j-sharded, fp8 DoubleRow).

Math (from the reference):
  loss = CE(logits, arange(B), sum)/B, logits[i,j] = -kl[i,j]/T
  kl[i,j] = hneg[j] - Li[i].Q[j], Q = softmax(c_j), hneg[j] = sum Q logQ.
  Per-row-i shifts cancel in log-softmax, so with E = exp(c_j),
  Z_j = sum E, A_j = sum E*c_j, and T = 1/2:
    G[i,j] = (c_i[i].Q_j)/T + 2 lnZ_j - 2 A_j/Z_j   (logits + per-i const)
    loss   = sum_i [logsumexp_j G[i,j] - G[i,i]] / B

Sharding: core c owns c_j rows [512c, 512c+512) (4 partition-tiles of 128 j)
and the FULL c_i as a host-transposed fp8 ciT.  Each core computes stripes
S'[j=128, i=512] = sum_k W8[j,k] ciT[k,i] with W8 = fp8(exp(cjT))
(DoubleRow, K=256/matmul).  The scalar engine forms u = exp(S'*scw + bb2)
with per-partition scale scw_j = 2/Z_j and bias bb2_j = -2A/Z - b0c and
writes the per-(t, chunk) partial tiles straight out as zp; the HOST
applies the per-j weight zn2_j = (Z_j/Z0)^2 and reduces over j in f64.

Perf structure (vs the 94us baseline):
  - all DRAM<->SBUF transfers are contiguous per-partition (128 descriptors
    of 2-8KB each) and ride ONE sync hwdge ring in strict priority order:
    first weights (cjt t0) + first i-chunks, then the rest.  First matmul
    lands ~9us instead of ~25us.
  - exp activations write fp8 weights directly (no DVE cast pass).
  - no on-device zn2 scale / accumulation: the 32 stripe exps write
    independent fp16 tiles, DMA'd out per chunk (4MB, host sums in f64).
  - scalar engine instruction order is hand-interleaved (engines run
    in-order) so weight exps beat stripe exps and nothing head-of-line
    blocks on a late cj load.

ciT rides in an eighth-major host layout [1024, 8192] (row e*128+p, col
kt*512+i') ROLLED per core so local i-chunk 0 is the core's own j-range:
the diagonal G_ii comes straight out of chunk-0 stripes via an eye mask
(gd = diag(S')*scw + bb2 = G_ii - 2lnZ - b0c), all SPMD-uniform.

Constants b0c/Z0 cancel exactly in the host reduction.
"""

import sys

for _p in ("/opt/trn_rl_repo",):
    if _p not in sys.path:
        sys.path.insert(0, _p)

import numpy as np
import ml_dtypes

import concourse.bass as bass
import concourse.bacc as bacc
import concourse.tile as tile
from concourse import mybir
from concourse import bass_utils

B = 4096
D = 2048
TEMP = 0.5
NCORES = 8
SHARD = B // NCORES  # 512 j-rows per core
NT = SHARD // 128  # 4 j partition-tiles
KT = D // 128  # 16 k partition-tiles
NCH = 8  # i chunks of 512

F32 = mybir.dt.float32
F16 = mybir.dt.float16
F8 = mybir.dt.float8e4
AF = mybir.ActivationFunctionType
OP = mybir.AluOpType
AX = mybir.AxisListType
PM = mybir.MatmulPerfMode

B0C = -2.0  # bias recenter: u = exp(G - 2lnZ - B0C) stays ~1 in fp16
Z0 = 3400.0  # host weight zn2 = (Z/Z0)^2 ~ 1

# stripe order: chunk-pairs (cranges), t-major inside each pair
CRANGES = [(0, 1), (2, 3), (4, 5), (6, 7)]


def build_kernel_body(tc, zp_ap, gd_ap, zo_ap, cit_ap, cj_ap, cjt_ap, eye_ap):
    nc = tc.nc
    from contextlib import ExitStack

    with ExitStack() as ctx:
        singles = ctx.enter_context(tc.tile_pool(name="singles", bufs=1))
        scr = ctx.enter_context(tc.tile_pool(name="scr", bufs=2))
        ups = ctx.enter_context(tc.tile_pool(name="ups", bufs=3))
        dps = ctx.enter_context(tc.tile_pool(name="dps", bufs=2))
        psS = ctx.enter_context(tc.tile_pool(name="psS", bufs=8, space="PSUM"))

        # resident tiles (per-partition bytes in comments)
        # NB: PE operand strides matter — kt-stride 4096B (cit8) / 512B (wt8)
        # replicate the baseline layouts that stream at 216ns/matmul; the
        # "natural" contiguous chunk-major layouts measured 512ns/matmul.
        cit8 = singles.tile([128, KT, NCH * 512], F8)  # 64KB kt-major
        cjt16 = singles.tile([128, NT, KT, 128], F16)  # 16KB t-major k-major
        et16 = singles.tile([128, NT, KT, 128], F16)  # 16KB exp(cjt) fp16
        wt8 = singles.tile([128, KT, NT * 128], F8)  # 8KB kt-major weights
        cjs = singles.tile([128, NT, D], F16)  # 16KB j-major
        E16s = singles.tile([128, NT, D], F16)  # 16KB j-major exp
        eye32 = singles.tile([128, 128], F32)
        Zc = singles.tile([128, NT], F32)
        Ac = singles.tile([128, NT], F32)
        rzc = singles.tile([128, NT], F32)
        scw = singles.tile([128, NT], F32)
        tmp1 = singles.tile([128, NT], F32)
        bb2 = singles.tile([128, NT], F32)
        gd = singles.tile([128, NT], F32)

        # ── loads: ONE sync hwdge ring, strict priority order ─────────────
        # Every transfer is contiguous per-partition on both sides, so each
        # dma_start is ~128 descriptors (cheap issue, full bus efficiency).
        def ld_cjt(t, a, b):  # cjt16[:, t, a:b, :] <- cols t*2048+a*128 ...
            nc.sync.dma_start(
                out=cjt16[:, t, a:b, :],
                in_=cjt_ap[:, 2048 * t + 128 * a : 2048 * t + 128 * b],
            )

        def ld_cit(eng, kt, a, b):  # cit8[:, kt, cols a:b] contiguous runs
            eng.dma_start(
                out=cit8[:, kt, 512 * a : 512 * b],
                in_=cit_ap[128 * kt : 128 * (kt + 1), 512 * a : 512 * b],
            )

        def ld_cj(eng, t):
            eng.dma_start(
                out=cjs[:, t], in_=cj_ap[128 * t : 128 * (t + 1), :]
            )

        # Load schedule.  Q1 = cit cols 0-1024 (chunks 0-1: gates stripes
        # 1-8), Q2 = cols 1024-2048 (chunks 2-3), H2 = cols 2048-4096.
        # The sync hwdge ring carries cjt + Q1-even kts + Q2 + H2 in strict
        # priority order; Q1-odd kts ride the gpsimd swdge queue in
        # parallel (dep-free, issued immediately) so the two DMA paths
        # drain the critical Q1 window together.  cj is gated (below,
        # after the wx section) so it can't steal early bus bandwidth.
        ld_cjt(0, 0, 8)
        ld_cjt(0, 8, 16)
        for kt in (1, 3, 5, 7, 9, 11, 13, 15):
            ld_cit(nc.gpsimd, kt, 0, 2)
        ld_cit(nc.sync, 0, 0, 2)
        ld_cit(nc.sync, 2, 0, 2)
        ld_cjt(1, 0, 16)
        ld_cit(nc.sync, 4, 0, 2)
        ld_cit(nc.sync, 6, 0, 2)
        ld_cjt(2, 0, 16)
        ld_cit(nc.sync, 8, 0, 2)
        ld_cit(nc.sync, 10, 0, 2)
        ld_cjt(3, 0, 16)
        ld_cit(nc.sync, 12, 0, 2)
        ld_cit(nc.sync, 14, 0, 2)
        for kt in range(KT):
            ld_cit(nc.sync, kt, 2, 4)
        for kt in range(KT):
            ld_cit(nc.sync, kt, 4, 8)
        nc.gpsimd.dma_start(out=eye32, in_=eye_ap)

        # weight exps in PE-consumption order; t0 split in halves so the
        # first matmuls can start after only 256KB of cjt has landed
        def wx(t, a, b):
            nc.scalar.activation(
                out=et16[:, t, a:b, :], in_=cjt16[:, t, a:b, :], func=AF.Exp
            )
            # cast + reshuffle: t-major fp16 -> kt-major fp8 (strided out)
            nc.vector.tensor_copy(
                out=wt8[:, a:b, 128 * t : 128 * (t + 1)],
                in_=et16[:, t, a:b, :],
            )

        wx(0, 0, 8)
        wx(0, 8, 16)
        for t in range(1, NT):
            wx(t, 0, 16)

        # cj loads ride gpsimd swdge, throttled behind weight progress:
        # a 1-element copy into each cjs slice creates a WAW dep on the cj
        # dma, and the copy itself reads wt8 (NB: emitted AFTER the wx
        # casts above so the read actually depends on them) — so the cj
        # transfers can't run early and steal bus bandwidth from the
        # critical cjt+H1 window on the sync ring.
        for t in range(NT):
            gate = 127 if t < 2 else 511  # t0/t1 after wt8-t0b, t2/t3 after wt8-t3
            nc.gpsimd.tensor_copy(
                out=cjs[:, t, 0:1], in_=wt8[:, 15:16, gate : gate + 1]
            )
            ld_cj(nc.gpsimd, t)

        # ── per-t bias chain: E=exp(cj) (accum Z), A=sum(E*cj) via fused
        # DVE tensor_tensor_reduce, then tiny [128,1] tensor_scalar ops.
        def bias_chain(t, prio=False):
            # priority applies ONLY to the scalar exp: hoisting the DVE ops
            # would head-of-line block the weight casts on the in-order DVE
            # while this chain waits for its data.
            if prio:
                with tc.high_priority():
                    nc.scalar.activation(
                        out=E16s[:, t], in_=cjs[:, t], func=AF.Exp,
                        accum_out=Zc[:, t : t + 1],
                    )
            else:
                nc.scalar.activation(
                    out=E16s[:, t], in_=cjs[:, t], func=AF.Exp,
                    accum_out=Zc[:, t : t + 1],
                )
            nc.vector.reciprocal(out=rzc[:, t : t + 1], in_=Zc[:, t : t + 1])
            nc.vector.tensor_scalar_mul(scw[:, t : t + 1], rzc[:, t : t + 1], 2.0)
            nc.vector.tensor_scalar_mul(tmp1[:, t : t + 1], rzc[:, t : t + 1], -2.0)
            prod = scr.tile([128, D], F16, tag="prod")
            nc.vector.tensor_mul(prod, E16s[:, t], cjs[:, t])
            nc.vector.tensor_reduce(
                out=Ac[:, t : t + 1], in_=prod, axis=AX.X, op=OP.add
            )
            nc.vector.tensor_scalar(
                out=bb2[:, t : t + 1], in0=Ac[:, t : t + 1],
                scalar1=tmp1[:, t : t + 1], scalar2=-float(B0C),
                op0=OP.mult, op1=OP.add,
            )

        # bias-t0 gates the first stripe exp (and with it the PSUM drain):
        # high priority on E16-t0 so it preempts wx-t2/t3 in the scalar
        # stream the moment cj-t0 lands.
        bias_chain(0, prio=True)
        bias_chain(1)

        # ── stripes ───────────────────────────────────────────────────────
        def stripe(t, c, u_c):
            S_ps = psS.tile([128, 512], F32, tag="s")
            for k2 in range(KT // 2):
                nc.tensor.matmul(
                    S_ps,
                    wt8[:, 2 * k2 : 2 * k2 + 2, 128 * t : 128 * (t + 1)],
                    cit8[:, 2 * k2 : 2 * k2 + 2, 512 * c : 512 * (c + 1)],
                    start=(k2 == 0),
                    stop=(k2 == KT // 2 - 1),
                    perf_mode=PM.DoubleRow,
                )
            if c == 0:
                # diagonal: G_ii - 2lnZ - b0c = diag(S')*scw + bb2
                junk = dps.tile([128, 128], F32, tag="junk")
                nc.vector.tensor_mul(
                    junk, S_ps[:, 128 * t : 128 * (t + 1)], eye32
                )
                dd = dps.tile([128, 1], F32, tag="dd")
                nc.vector.tensor_reduce(out=dd, in_=junk, axis=AX.X, op=OP.add)
                nc.vector.tensor_scalar(
                    out=gd[:, t : t + 1], in0=dd,
                    scalar1=scw[:, t : t + 1], scalar2=bb2[:, t : t + 1],
                    op0=OP.mult, op1=OP.add,
                )
            nc.scalar.activation(
                out=u_c[:, t], in_=S_ps, func=AF.Exp,
                scale=scw[:, t : t + 1], bias=bb2[:, t : t + 1],
            )

        for icr, cr in enumerate(CRANGES):
            u_tiles = {
                c: ups.tile([128, NT, 512], F16, tag="u", name=f"u{icr}_{c}")
                for c in cr
            }
            for t in range(NT):
                # interleave remaining bias chains so the in-order scalar
                # stream never blocks stripe exps on a late cj load
                if icr == 0 and t == 1:
                    bias_chain(2)
                if icr == 0 and t == 2:
                    bias_chain(3)
                for c in cr:
                    stripe(t, c, u_tiles[c])
                    # spread the output writes: chunk done after its t3 X;
                    # for the final chunk split t0-2/t3 to shorten the tail
                    last = (icr, c) == (len(CRANGES) - 1, cr[-1])
                    if last and t == NT - 2:
                        nc.sync.dma_start(
                            out=zp_ap[:, 2048 * c : 2048 * c + 1536],
                            in_=u_tiles[c][:, 0:3, :],
                        )
                    if t == NT - 1:
                        if last:
                            nc.sync.dma_start(
                                out=zp_ap[:, 2048 * c + 1536 : 2048 * (c + 1)],
                                in_=u_tiles[c][:, 3:4, :],
                            )
                        else:
                            nc.sync.dma_start(
                                out=zp_ap[:, 2048 * c : 2048 * (c + 1)],
                                in_=u_tiles[c],
                            )

        nc.sync.dma_start(out=gd_ap, in_=gd)
        nc.sync.dma_start(out=zo_ap, in_=Zc)


_NC_CACHE = {}


def build_nc():
    key = "nc"
    if key in _NC_CACHE:
        return _NC_CACHE[key]
    nc = bacc.Bacc("TRN2", target_bir_lowering=False, debug=False)
    cit = nc.dram_tensor("cit", [KT * 128, NCH * 512], F8, kind="ExternalInput").ap()
    cj = nc.dram_tensor("cj", [SHARD, D], F16, kind="ExternalInput").ap()
    cjt = nc.dram_tensor("cjt", [128, NT * KT * 128], F16, kind="ExternalInput").ap()
    eye = nc.dram_tensor("eye", [128, 128], F32, kind="ExternalInput").ap()
    zp = nc.dram_tensor("zp", [128, NCH * NT * 512], F16, kind="ExternalOutput").ap()
    gd = nc.dram_tensor("gd", [128, NT], F32, kind="ExternalOutput").ap()
    zo = nc.dram_tensor("zo", [128, NT], F32, kind="ExternalOutput").ap()
    with tile.TileContext(nc) as tc:
        build_kernel_body(tc, zp, gd, zo, cit, cj, cjt, eye)
    nc.compile()
    _NC_CACHE[key] = nc
    return nc


def make_in_maps(c_i, c_j):
    # kt-major ciT [D, B]: row kt*128+p, col = i-chunks ROLLED per core so
    # local chunk 0 = the core's own j-range (the diagonal block).
    cit8 = c_i.T.astype(ml_dtypes.float8_e4m3)  # [D, B]
    base = cit8.reshape(D, NCH, 512)
    eye = np.eye(128, dtype=np.float32)
    in_maps = []
    for c in range(NCORES):
        rolled = np.ascontiguousarray(np.roll(base, -c, axis=1)).reshape(D, B)
        cjsh = c_j[SHARD * c : SHARD * (c + 1)].astype(np.float16)
        in_maps.append(
            {
                "cit": rolled,
                "cj": cjsh,
                # t-major k-major transposed shard: [p, t, kt, j']
                "cjt": np.ascontiguousarray(
                    cjsh.T.reshape(KT, 128, NT, 128).transpose(1, 2, 0, 3)
                ).reshape(128, NT * KT * 128),
                "eye": eye,
            }
        )
    return in_maps


def kernel(c_i, c_j, **kwargs):
    c_i = np.ascontiguousarray(np.asarray(c_i, dtype=np.float32))
    c_j = np.ascontiguousarray(np.asarray(c_j, dtype=np.float32))
    nc = build_nc()
    in_maps = make_in_maps(c_i, c_j)
    res = bass_utils.run_bass_kernel_spmd(
        nc, in_maps, core_ids=list(range(NCORES))
    )

    Zi = np.zeros(B, dtype=np.float64)
    gii_sum = np.float64(0.0)
    for c, r in enumerate(res.results):
        zo = r["zo"].astype(np.float64)  # [128, NT] = Z_j
        w = (zo / Z0) ** 2  # host-applied zn2
        zp = r["zp"].astype(np.float64).reshape(128, NCH, NT, 512)
        zl = np.einsum("pt,pcti->ci", w, zp)  # [NCH, 512]
        Zi += np.roll(zl, c, axis=0).reshape(-1)
        # G_ii = gd + 2 lnZ + b0c  (gd = diag*scw + bb2 lacks the 2 lnZ)
        gii_sum += (r["gd"].astype(np.float64) + 2.0 * np.log(zo) + B0C).sum()
    lse_sum = np.log(Zi).sum() + B * (B0C + 2.0 * np.log(Z0))
    loss = (lse_sum - gii_sum) / B
    return np.float32(loss).reshape(())


# revision 35
# speedup vs baseline: 1.0761x; 1.0761x over previous
"""ClusterKLLoss Trainium2 kernel (8 NeuronCores, j-sharded, fp8 DoubleRow).

Math (from the reference):
  loss = CE(logits, arange(B), sum)/B, logits[i,j] = -kl[i,j]/T
  kl[i,j] = hneg[j] - Li[i].Q[j], Q = softmax(c_j), hneg[j] = sum Q logQ.
  Per-row-i shifts cancel in log-softmax, so with E = exp(c_j),
  Z_j = sum E, A_j = sum E*c_j, and T = 1/2:
    G[i,j] = (c_i[i].Q_j)/T + 2 lnZ_j - 2 A_j/Z_j   (logits + per-i const)
    loss   = sum_i [logsumexp_j G[i,j] - G[i,i]] / B

Sharding: core c owns c_j rows [512c, 512c+512) (4 partition-tiles of 128 j)
and the FULL c_i as a host-transposed fp8 ciT.  Each core computes stripes
S'[j=128, i=512] = sum_k W8[j,k] ciT[k,i] with W8 = fp8(exp(cjT))
(DoubleRow, K=256/matmul).  The scalar engine forms u = exp(S'*scw + bb2)
with per-partition scale scw_j = 2/Z_j and bias bb2_j = -2A/Z - b0c and
writes the per-(t, chunk) partial tiles straight out as zp; the HOST
applies the per-j weight zn2_j = (Z_j/Z0)^2 and reduces over j in f64.

Perf structure (vs the 94us baseline):
  - all DRAM<->SBUF transfers are contiguous per-partition (128 descriptors
    of 2-8KB each) and ride ONE sync hwdge ring in strict priority order:
    first weights (cjt t0) + first i-chunks, then the rest.  First matmul
    lands ~9us instead of ~25us.
  - exp activations write fp8 weights directly (no DVE cast pass).
  - no on-device zn2 scale / accumulation: the 32 stripe exps write
    independent fp16 tiles, DMA'd out per chunk (4MB, host sums in f64).
  - scalar engine instruction order is hand-interleaved (engines run
    in-order) so weight exps beat stripe exps and nothing head-of-line
    blocks on a late cj load.

ciT rides in an eighth-major host layout [1024, 8192] (row e*128+p, col
kt*512+i') ROLLED per core so local i-chunk 0 is the core's own j-range:
the diagonal G_ii comes straight out of chunk-0 stripes via an eye mask
(gd = diag(S')*scw + bb2 = G_ii - 2lnZ - b0c), all SPMD-uniform.

Constants b0c/Z0 cancel exactly in the host reduction.
"""

import sys

for _p in ("/opt/trn_rl_repo",):
    if _p not in sys.path:
        sys.path.insert(0, _p)

import numpy as np
import ml_dtypes

import concourse.bass as bass
import concourse.bacc as bacc
import concourse.tile as tile
from concourse import mybir
from concourse import bass_utils

B = 4096
D = 2048
TEMP = 0.5
NCORES = 8
SHARD = B // NCORES  # 512 j-rows per core
NT = SHARD // 128  # 4 j partition-tiles
KT = D // 128  # 16 k partition-tiles
NCH = 8  # i chunks of 512

F32 = mybir.dt.float32
F16 = mybir.dt.float16
F8 = mybir.dt.float8e4
AF = mybir.ActivationFunctionType
OP = mybir.AluOpType
AX = mybir.AxisListType
PM = mybir.MatmulPerfMode

B0C = -2.0  # bias recenter: u = exp(G - 2lnZ - B0C) stays ~1 in fp16
Z0 = 3400.0  # host weight zn2 = (Z/Z0)^2 ~ 1

# stripe order: chunk-pairs (cranges), t-major inside each pair
CRANGES = [(0, 1), (2, 3), (4, 5), (6, 7)]


def build_kernel_body(tc, zp_ap, gd_ap, zo_ap, cit_ap, cj_ap, cjt_ap, eye_ap):
    nc = tc.nc
    from contextlib import ExitStack

    with ExitStack() as ctx:
        singles = ctx.enter_context(tc.tile_pool(name="singles", bufs=1))
        scr = ctx.enter_context(tc.tile_pool(name="scr", bufs=2))
        ups = ctx.enter_context(tc.tile_pool(name="ups", bufs=3))
        dps = ctx.enter_context(tc.tile_pool(name="dps", bufs=2))
        psS = ctx.enter_context(tc.tile_pool(name="psS", bufs=8, space="PSUM"))

        # resident tiles (per-partition bytes in comments)
        # NB: PE operand strides matter — kt-stride 4096B (cit8) / 512B (wt8)
        # replicate the baseline layouts that stream at 216ns/matmul; the
        # "natural" contiguous chunk-major layouts measured 512ns/matmul.
        cit8 = singles.tile([128, KT, NCH * 512], F8)  # 64KB kt-major
        cjt16 = singles.tile([128, NT, KT, 128], F16)  # 16KB t-major k-major
        et16 = singles.tile([128, NT, KT, 128], F16)  # 16KB exp(cjt) fp16
        wt8 = singles.tile([128, KT, NT * 128], F8)  # 8KB kt-major weights
        cjs = singles.tile([128, NT, D], F16)  # 16KB j-major
        E16s = singles.tile([128, NT, D], F16)  # 16KB j-major exp
        eye32 = singles.tile([128, 128], F32)
        Zc = singles.tile([128, NT], F32)
        Ac = singles.tile([128, NT], F32)
        rzc = singles.tile([128, NT], F32)
        scw = singles.tile([128, NT], F32)
        tmp1 = singles.tile([128, NT], F32)
        bb2 = singles.tile([128, NT], F32)
        gd = singles.tile([128, NT], F32)

        # ── loads: ONE sync hwdge ring, strict priority order ─────────────
        # Every transfer is contiguous per-partition on both sides, so each
        # dma_start is ~128 descriptors (cheap issue, full bus efficiency).
        def ld_cjt(t, a, b):  # cjt16[:, t, a:b, :] <- cols t*2048+a*128 ...
            nc.sync.dma_start(
                out=cjt16[:, t, a:b, :],
                in_=cjt_ap[:, 2048 * t + 128 * a : 2048 * t + 128 * b],
            )

        def ld_cit(eng, kt, a, b):  # cit8[:, kt, cols a:b] contiguous runs
            eng.dma_start(
                out=cit8[:, kt, 512 * a : 512 * b],
                in_=cit_ap[128 * kt : 128 * (kt + 1), 512 * a : 512 * b],
            )

        def ld_cj(eng, t):
            eng.dma_start(
                out=cjs[:, t], in_=cj_ap[128 * t : 128 * (t + 1), :]
            )

        # Load schedule: ONE sync hwdge ring in strict priority order —
        # a single sequential DRAM stream measures ~370GB/s while two
        # parallel queues thrash down to ~250GB/s, so bulk stays on one
        # ring.  H1 = cit cols 0-2048 (chunks 0-3), H2 = cols 2048-4096.
        # cj is gated (below, after the wx section) so it can't steal
        # early bus bandwidth from the critical cjt+H1 window.
        ld_cjt(0, 0, 8)
        ld_cjt(0, 8, 16)
        ld_cit(nc.sync, 0, 0, 4)
        ld_cit(nc.sync, 1, 0, 4)
        ld_cit(nc.sync, 2, 0, 4)
        ld_cit(nc.sync, 3, 0, 4)
        ld_cjt(1, 0, 16)
        ld_cit(nc.sync, 4, 0, 4)
        ld_cit(nc.sync, 5, 0, 4)
        ld_cjt(2, 0, 16)
        ld_cit(nc.sync, 6, 0, 4)
        ld_cit(nc.sync, 7, 0, 4)
        ld_cjt(3, 0, 16)
        for kt in range(8, KT):
            ld_cit(nc.sync, kt, 0, 4)
        for kt in range(KT):
            ld_cit(nc.sync, kt, 4, 8)
        nc.gpsimd.dma_start(out=eye32, in_=eye_ap)

        # weight exps in PE-consumption order; t0 split in halves so the
        # first matmuls can start after only 256KB of cjt has landed
        def wx(t, a, b):
            nc.scalar.activation(
                out=et16[:, t, a:b, :], in_=cjt16[:, t, a:b, :], func=AF.Exp
            )
            # cast + reshuffle: t-major fp16 -> kt-major fp8 (strided out)
            nc.vector.tensor_copy(
                out=wt8[:, a:b, 128 * t : 128 * (t + 1)],
                in_=et16[:, t, a:b, :],
            )

        wx(0, 0, 8)
        wx(0, 8, 16)
        for t in range(1, NT):
            wx(t, 0, 16)

        # cj loads ride gpsimd swdge, throttled behind weight progress:
        # a 1-element copy into each cjs slice creates a WAW dep on the cj
        # dma, and the copy itself reads wt8 (NB: emitted AFTER the wx
        # casts above so the read actually depends on them) — so the cj
        # transfers can't run early and steal bus bandwidth from the
        # critical cjt+H1 window on the sync ring.
        for t in range(NT):
            gate = 127 if t < 2 else 511  # t0/t1 after wt8-t0b, t2/t3 after wt8-t3
            nc.gpsimd.tensor_copy(
                out=cjs[:, t, 0:1], in_=wt8[:, 15:16, gate : gate + 1]
            )
            ld_cj(nc.gpsimd, t)

        # ── per-t bias chain: E=exp(cj) (accum Z), A=sum(E*cj) via fused
        # DVE tensor_tensor_reduce, then tiny [128,1] tensor_scalar ops.
        def bias_chain(t, prio=False):
            # priority applies ONLY to the scalar exp: hoisting the DVE ops
            # would head-of-line block the weight casts on the in-order DVE
            # while this chain waits for its data.
            if prio:
                with tc.high_priority():
                    nc.scalar.activation(
                        out=E16s[:, t], in_=cjs[:, t], func=AF.Exp,
                        accum_out=Zc[:, t : t + 1],
                    )
            else:
                nc.scalar.activation(
                    out=E16s[:, t], in_=cjs[:, t], func=AF.Exp,
                    accum_out=Zc[:, t : t + 1],
                )
            nc.vector.reciprocal(out=rzc[:, t : t + 1], in_=Zc[:, t : t + 1])
            nc.vector.tensor_scalar_mul(scw[:, t : t + 1], rzc[:, t : t + 1], 2.0)
            nc.vector.tensor_scalar_mul(tmp1[:, t : t + 1], rzc[:, t : t + 1], -2.0)
            prod = scr.tile([128, D], F16, tag="prod")
            nc.vector.tensor_mul(prod, E16s[:, t], cjs[:, t])
            nc.vector.tensor_reduce(
                out=Ac[:, t : t + 1], in_=prod, axis=AX.X, op=OP.add
            )
            nc.vector.tensor_scalar(
                out=bb2[:, t : t + 1], in0=Ac[:, t : t + 1],
                scalar1=tmp1[:, t : t + 1], scalar2=-float(B0C),
                op0=OP.mult, op1=OP.add,
            )

        # bias-t0 gates the first stripe exp (and with it the PSUM drain):
        # high priority on E16-t0 so it preempts wx-t2/t3 in the scalar
        # stream the moment cj-t0 lands.
        bias_chain(0, prio=True)
        bias_chain(1)

        # ── stripes ───────────────────────────────────────────────────────
        def stripe(t, c, u_c):
            S_ps = psS.tile([128, 512], F32, tag="s")
            for k2 in range(KT // 2):
                nc.tensor.matmul(
                    S_ps,
                    wt8[:, 2 * k2 : 2 * k2 + 2, 128 * t : 128 * (t + 1)],
                    cit8[:, 2 * k2 : 2 * k2 + 2, 512 * c : 512 * (c + 1)],
                    start=(k2 == 0),
                    stop=(k2 == KT // 2 - 1),
                    perf_mode=PM.DoubleRow,
                )
            if c == 0:
                # diagonal: G_ii - 2lnZ - b0c = diag(S')*scw + bb2
                junk = dps.tile([128, 128], F32, tag="junk")
                nc.vector.tensor_mul(
                    junk, S_ps[:, 128 * t : 128 * (t + 1)], eye32
                )
                dd = dps.tile([128, 1], F32, tag="dd")
                nc.vector.tensor_reduce(out=dd, in_=junk, axis=AX.X, op=OP.add)
                nc.vector.tensor_scalar(
                    out=gd[:, t : t + 1], in0=dd,
                    scalar1=scw[:, t : t + 1], scalar2=bb2[:, t : t + 1],
                    op0=OP.mult, op1=OP.add,
                )
            nc.scalar.activation(
                out=u_c[:, t], in_=S_ps, func=AF.Exp,
                scale=scw[:, t : t + 1], bias=bb2[:, t : t + 1],
            )

        for icr, cr in enumerate(CRANGES):
            u_tiles = {
                c: ups.tile([128, NT, 512], F16, tag="u", name=f"u{icr}_{c}")
                for c in cr
            }
            for t in range(NT):
                # interleave remaining bias chains so the in-order scalar
                # stream never blocks stripe exps on a late cj load
                if icr == 0 and t == 1:
                    bias_chain(2)
                if icr == 0 and t == 2:
                    bias_chain(3)
                for c in cr:
                    stripe(t, c, u_tiles[c])
                    # spread the output writes: chunk done after its t3 X;
                    # for the final chunk split t0-2/t3 to shorten the tail
                    last = (icr, c) == (len(CRANGES) - 1, cr[-1])
                    if last and t == NT - 2:
                        nc.sync.dma_start(
                            out=zp_ap[:, 2048 * c : 2048 * c + 1536],
                            in_=u_tiles[c][:, 0:3, :],
                        )
                    if t == NT - 1:
                        if last:
                            nc.sync.dma_start(
                                out=zp_ap[:, 2048 * c + 1536 : 2048 * (c + 1)],
                                in_=u_tiles[c][:, 3:4, :],
                            )
                        else:
                            nc.sync.dma_start(
                                out=zp_ap[:, 2048 * c : 2048 * (c + 1)],
                                in_=u_tiles[c],
                            )

        nc.sync.dma_start(out=gd_ap, in_=gd)
        nc.sync.dma_start(out=zo_ap, in_=Zc)


_NC_CACHE = {}


def build_nc():
    key = "nc"
    if key in _NC_CACHE:
        return _NC_CACHE[key]
    nc = bacc.Bacc("TRN2", target_bir_lowering=False, debug=False)
    cit = nc.dram_tensor("cit", [KT * 128, NCH * 512], F8, kind="ExternalInput").ap()
    cj = nc.dram_tensor("cj", [SHARD, D], F16, kind="ExternalInput").ap()
    cjt = nc.dram_tensor("cjt", [128, NT * KT * 128], F16, kind="ExternalInput").ap()
    eye = nc.dram_tensor("eye", [128, 128], F32, kind="ExternalInput").ap()
    zp = nc.dram_tensor("zp", [128, NCH * NT * 512], F16, kind="ExternalOutput").ap()
    gd = nc.dram_tensor("gd", [128, NT], F32, kind="ExternalOutput").ap()
    zo = nc.dram_tensor("zo", [128, NT], F32, kind="ExternalOutput").ap()
    with tile.TileContext(nc) as tc:
        build_kernel_body(tc, zp, gd, zo, cit, cj, cjt, eye)
    nc.compile()
    _NC_CACHE[key] = nc
    return nc


def make_in_maps(c_i, c_j):
    # kt-major ciT [D, B]: row kt*128+p, col = i-chunks ROLLED per core so
    # local chunk 0 = the core's own j-range (the diagonal block).
    cit8 = c_i.T.astype(ml_dtypes.float8_e4m3)  # [D, B]
    base = cit8.reshape(D, NCH, 512)
    eye = np.eye(128, dtype=np.float32)
    in_maps = []
    for c in range(NCORES):
        rolled = np.ascontiguousarray(np.roll(base, -c, axis=1)).reshape(D, B)
        cjsh = c_j[SHARD * c : SHARD * (c + 1)].astype(np.float16)
        in_maps.append(
            {
                "cit": rolled,
                "cj": cjsh,
                # t-major k-major transposed shard: [p, t, kt, j']
                "cjt": np.ascontiguousarray(
                    cjsh.T.reshape(KT, 128, NT, 128).transpose(1, 2, 0, 3)
                ).reshape(128, NT * KT * 128),
                "eye": eye,
            }
        )
    return in_maps


def kernel(c_i, c_j, **kwargs):
    c_i = np.ascontiguousarray(np.asarray(c_i, dtype=np.float32))
    c_j = np.ascontiguousarray(np.asarray(c_j, dtype=np.float32))
    nc = build_nc()
    in_maps = make_in_maps(c_i, c_j)
    res = bass_utils.run_bass_kernel_spmd(
        nc, in_maps, core_ids=list(range(NCORES))
    )

    Zi = np.zeros(B, dtype=np.float64)
    gii_sum = np.float64(0.0)
    for c, r in enumerate(res.results):
        zo = r["zo"].astype(np.float64)  # [128, NT] = Z_j
        w = (zo / Z0) ** 2  # host-applied zn2
        zp = r["zp"].astype(np.float64).reshape(128, NCH, NT, 512)
        zl = np.einsum("pt,pcti->ci", w, zp)  # [NCH, 512]
        Zi += np.roll(zl, c, axis=0).reshape(-1)
        # G_ii = gd + 2 lnZ + b0c  (gd = diag*scw + bb2 lacks the 2 lnZ)
        gii_sum += (r["gd"].astype(np.float64) + 2.0 * np.log(zo) + B0C).sum()
    lse_sum = np.log(Zi).sum() + B * (B0C + 2.0 * np.log(Z0))
    loss = (lse_sum - gii_sum) / B
    return np.float32(loss).reshape(())
